# revision 9
# baseline (speedup 1.0000x reference)
"""DTIGCN message-passing kernel for 8 Trainium2 NeuronCores.

Strategy (per spec sharding hint): shard every adjacency matrix row-wise
(destination dim) across the 8 cores; replicate the [64,64] weights and
source features. Each core computes rn(A_shard) @ act(X @ W^T + b) for
its rows plus the local mean+l2norm — no cross-core reduction needed.

Key layout choices (host-side prep, done inside kernel()):
  - A shards are shipped TRANSPOSED and swizzled ([S/512, 128, 4*R],
    fp16) so the device loads tiles with the contraction (source) dim on
    SBUF partitions in 4KB-contiguous DMA runs — no on-chip transposes
    of the big matrices, near-peak HBM bandwidth.
  - Source dims are zero-padded to multiples of 512 (padded A columns
    are 0, padded features are 0) so every 128-row block is full.
  - Features are shipped transposed with a ones row appended ([65, S]),
    and weights as [W^T; b] packs (one pack per source type), so
    H = act(X@W^T + b) is one fp32 matmul per 128-row block per source.
  - H gets a ones column appended: the main matmul H_aug^T @ A_shard^T
    yields both (A@H)^T and the row sums (for row normalization).
  - The main product is computed transposed (psum [65, R]) so the big
    operand (A^T tile) is the moving operand: one matmul per 128-source
    block per relation. A cheap PE transpose (identity trick) brings
    each [65, 128] chunk back to natural orientation for the epilogue.
  - elu is computed as max(z,0) + exp(min(z,0))  (i.e. elu + 1); the -1
    is recovered for free because rn(A) rows sum to exactly 1, by
    subtracting the count of elu-messages per destination at the end.
  - mean folds into the l2 norm: l2norm(s/n) = s / max(||s||, n*eps).
"""

import os
import numpy as np

ND, NPR, NDI, NSE, D = 4000, 4000, 3000, 2000, 64
N_CORES = 8

# relations of each source type; relu relation first, then elu relations
SRC_PACKS = {
    "drug": ["dd", "pd", "did", "sed"],
    "protein": ["dp", "pp", "dip"],
    "disease": ["ddi", "pdi"],
    "sideeffect": ["dse"],
}
REL_DEST = {
    "dd": "drug", "dp": "drug", "ddi": "drug", "dse": "drug",
    "pp": "protein", "pd": "protein", "pdi": "protein",
    "did": "disease", "dip": "disease", "sed": "sideeffect",
}
REL_RELU = {r: r in ("dd", "dp", "ddi", "dse") for r in REL_DEST}
TYPE_N = {"drug": ND, "protein": NPR, "disease": NDI, "sideeffect": NSE}
TYPE_SPAD = {t: ((n + 511) // 512) * 512 for t, n in TYPE_N.items()}
TYPES = ["drug", "protein", "disease", "sideeffect"]
FEAT_KEY = {t: f"feat_{t}" for t in TYPES}
N_MEAN = {"drug": 5, "protein": 4, "disease": 3, "sideeffect": 2}
N_ELU_DEST = {"drug": 0, "protein": 3, "disease": 2, "sideeffect": 1}
MAIN_ORDER = ["dd", "dp", "ddi", "dse", "pp", "pd", "pdi", "did", "dip", "sed"]
REL_SRC = {r: t for t, rels in SRC_PACKS.items() for r in rels}
G_SW = 2  # swizzle groups (of 4 blocks = 512 source rows) per AT dma


def _ceil_div(a, b):
    return (a + b - 1) // b


def _split_sync_waits(nc, max_waits=1):
    """Hoist excess per-instruction sem waits onto preceding NOPs.

    The staged walrus build rejects >max_waits sync waits on a single
    instruction ("Too many sync wait commands" in
    CoreV3GenImpl::setupSyncWait). Each engine executes its stream in
    order, so satisfying the extra waits on same-engine NOPs emitted
    immediately before the instruction is semantically identical.
    """
    import bass_rust

    cur_bb = nc.cur_bb
    assert cur_bb is not None
    host_list = cur_bb.bb.instructions

    def make_nop(engine, wait):
        bi = nc.engines[engine].nop()
        # un-append it from the current bb; we place it manually below
        assert host_list and host_list[-1] is bi.ins
        host_list.pop()
        bi.ins.sync_info = bass_rust.SyncInfo(on_wait=[wait], on_update=[])
        return bi.ins

    for f in nc.m.functions:
        for bb in f.blocks:
            new_insts = []
            changed = False
            for inst in bb.instructions:
                si = inst.sync_info
                waits = list(si.on_wait) if si and si.on_wait else []
                if len(waits) > max_waits:
                    for w in waits[max_waits:]:
                        new_insts.append(make_nop(inst.engine, w))
                    inst.sync_info = bass_rust.SyncInfo(
                        on_wait=waits[:max_waits],
                        on_update=list(si.on_update) if si.on_update else [],
                    )
                    changed = True
                new_insts.append(inst)
            if changed:
                bb.instructions = new_insts


def build_nc():
    """Build the per-core Bass program (identical across the 8 cores)."""
    from contextlib import ExitStack

    import concourse.bass as bass
    import concourse.mybir as mybir
    import concourse.tile as tile
    from concourse.masks import make_identity

    f32, f16 = mybir.dt.float32, mybir.dt.float16
    AF = mybir.ActivationFunctionType
    OP = mybir.AluOpType

    nc = bass.Bass()
    at_dram = {}
    for r in MAIN_ORDER:
        SP = TYPE_SPAD[REL_SRC[r]]
        R = TYPE_N[REL_DEST[r]] // N_CORES
        at_dram[r] = nc.dram_tensor(
            f"at_{r}", [SP // 512, 128, 4 * R], f16, kind="ExternalInput"
        )
    xaugt_dram = {
        t: nc.dram_tensor(
            f"xaugt_{t}", [D + 1, TYPE_SPAD[t]], f32, kind="ExternalInput"
        )
        for t in TYPES
    }
    wpack_dram = {
        t: nc.dram_tensor(
            f"wpack_{t}", [D + 1, D * len(SRC_PACKS[t])], f32, kind="ExternalInput"
        )
        for t in TYPES
    }
    featd_dram = {
        t: nc.dram_tensor(
            f"featd_{t}", [TYPE_N[t] // N_CORES, D], f32, kind="ExternalInput"
        )
        for t in TYPES
    }
    out_dram = {
        t: nc.dram_tensor(
            f"out_{t}", [TYPE_N[t] // N_CORES, D], f32, kind="ExternalOutput"
        )
        for t in TYPES
    }

    with tile.TileContext(nc) as tc, ExitStack() as ctx:
        persist = ctx.enter_context(tc.tile_pool(name="persist", bufs=1))
        atpool = ctx.enter_context(tc.tile_pool(name="atpool", bufs=6))
        etmp = ctx.enter_context(tc.tile_pool(name="etmp", bufs=3))
        outp = ctx.enter_context(tc.tile_pool(name="outp", bufs=4))
        hpsum = ctx.enter_context(tc.tile_pool(name="hpsum", bufs=2, space="PSUM"))
        opsum = ctx.enter_context(tc.tile_pool(name="opsum", bufs=4, space="PSUM"))
        tpsum = ctx.enter_context(tc.tile_pool(name="tpsum", bufs=2, space="PSUM"))

        ident = persist.tile([D + 1, D + 1], f32, name="ident")
        make_identity(nc, ident)

        # ---- persistent loads -------------------------------------------
        xt_sb, wp_sb, haug, acc = {}, {}, {}, {}
        for t in TYPES:
            SP, k = TYPE_SPAD[t], len(SRC_PACKS[t])
            nblk = SP // 128
            xt = persist.tile([D + 1, SP], f32, name=f"xt_{t}")
            nc.scalar.dma_start(out=xt, in_=xaugt_dram[t][:, :])
            xt_sb[t] = xt
            wp = persist.tile([D + 1, D * k], f32, name=f"wp_{t}")
            nc.scalar.dma_start(out=wp, in_=wpack_dram[t][:, :])
            wp_sb[t] = wp
            hg = persist.tile([128, nblk, k, D + 1], f16, name=f"haug_{t}")
            nc.vector.memset(hg[:, :, :, D:D + 1], 1.0)  # ones column
            haug[t] = hg

            R = TYPE_N[t] // N_CORES
            nrc = _ceil_div(R, 128)
            ac = persist.tile([128, nrc, D], f32, name=f"acc_{t}")
            for rc in range(nrc):
                rcw = min(128, R - rc * 128)
                nc.scalar.dma_start(
                    out=ac[0:rcw, rc, :],
                    in_=featd_dram[t][rc * 128 : rc * 128 + rcw, :],
                )
            acc[t] = ac

        # ---- H phase: H_aug = [act(X@W^T + b) | 1] per source type ------
        for t in TYPES:
            SP, k = TYPE_SPAD[t], len(SRC_PACKS[t])
            nblk = SP // 128
            ne = k - 1  # number of elu relations in this pack
            for b in range(nblk):
                ph = hpsum.tile([128, D * k], f32, name="ph", tag="ph")
                nc.tensor.matmul(
                    ph,
                    lhsT=xt_sb[t][:, b * 128 : (b + 1) * 128],
                    rhs=wp_sb[t][:, :],
                    start=True,
                    stop=True,
                )
                nc.scalar.activation(haug[t][:, b, 0, 0:D], ph[:, 0:D], AF.Relu)
                if ne:
                    w = D * ne
                    zmin = etmp.tile([128, D * 3], f32, name="zmin", tag="zmin")
                    zexp = etmp.tile([128, D * 3], f32, name="zexp", tag="zexp")
                    nc.vector.tensor_scalar(
                        zmin[:, 0:w], ph[:, D : D + w], 0.0, None, OP.min
                    )
                    nc.scalar.activation(zexp[:, 0:w], zmin[:, 0:w], AF.Exp)
                    # H' = max(z,0) + exp(min(z,0)), fused in one DVE op
                    nc.vector.scalar_tensor_tensor(
                        out=haug[t][:, b, 1:k, 0:D],
                        in0=ph[:, D : D + w].rearrange("p (j f) -> p j f", f=D),
                        scalar=0.0,
                        in1=zexp[:, 0:w].rearrange("p (j f) -> p j f", f=D),
                        op0=OP.max,
                        op1=OP.add,
                    )

        # ---- main phase: per relation, (A_shard @ H_aug)^T in PSUM ------
        for r in MAIN_ORDER:
            t, dst = REL_SRC[r], REL_DEST[r]
            j = SRC_PACKS[t].index(r)
            SP = TYPE_SPAD[t]
            R = TYPE_N[dst] // N_CORES
            nrc = _ceil_div(R, 128)
            nsw = SP // 512  # swizzle groups
            nblk = SP // 128

            pot = opsum.tile([D + 1, R], f32, name=f"pot_{r}", tag="pot")
            for sg0 in range(0, nsw, G_SW):
                gw = min(G_SW, nsw - sg0)
                atb = atpool.tile([128, G_SW, 4 * R], f16, name="atb", tag="atb")
                nc.sync.dma_start(
                    out=atb[:, 0:gw, :],
                    in_=at_dram[r][sg0 : sg0 + gw, :, :].rearrange(
                        "s p w -> p s w"
                    ),
                )
                for sgi in range(gw):
                    for g4 in range(4):
                        b = (sg0 + sgi) * 4 + g4
                        nc.tensor.matmul(
                            pot,
                            lhsT=haug[t][:, b, j, :],
                            rhs=atb[:, sgi, g4 * R : (g4 + 1) * R],
                            start=(b == 0),
                            stop=(b == nblk - 1),
                        )

            # transpose back to natural [R, 65] in 128-row chunks, then
            # acc += (A@H) * (1/rowsum); the elu "-1" is deferred
            ot = etmp.tile([D + 1, R], f32, name="ot", tag="ot")
            nc.vector.tensor_copy(ot, pot)
            for rc in range(nrc):
                rcw = min(128, R - rc * 128)
                ptp = tpsum.tile([128, D + 1], f32, name="ptp", tag="ptp")
                nc.tensor.transpose(
                    ptp[0:rcw, :],
                    ot[:, rc * 128 : rc * 128 + rcw],
                    ident,
                )
                rp = etmp.tile([128, 1], f32, name="rp", tag="rp")
                nc.vector.reciprocal(rp[0:rcw, :], ptp[0:rcw, D : D + 1])
                nc.vector.scalar_tensor_tensor(
                    out=acc[dst][0:rcw, rc, :],
                    in0=ptp[0:rcw, 0:D],
                    scalar=rp[0:rcw, :],
                    in1=acc[dst][0:rcw, rc, :],
                    op0=OP.mult,
                    op1=OP.add,
                )

        # ---- final: subtract elu count, l2-normalize (mean folds in) ----
        for t in TYPES:
            R = TYPE_N[t] // N_CORES
            nrc = _ceil_div(R, 128)
            n_elu = N_ELU_DEST[t]
            n_mean = N_MEAN[t]
            for rc in range(nrc):
                rcw = min(128, R - rc * 128)
                if n_elu:
                    am = outp.tile([128, D], f32, name="am", tag="am")
                    nc.vector.tensor_scalar(
                        am[0:rcw, :], acc[t][0:rcw, rc, :], float(n_elu), None,
                        OP.subtract,
                    )
                    amv = am[0:rcw, :]
                else:
                    amv = acc[t][0:rcw, rc, :]
                sq = etmp.tile([128, D], f32, name="sq", tag="sq")
                ss = etmp.tile([128, 1], f32, name="ss", tag="ss")
                nc.scalar.activation(
                    sq[0:rcw, :], amv, AF.Square, accum_out=ss[0:rcw, :]
                )
                nrm = etmp.tile([128, 1], f32, name="nrm", tag="nrm")
                nc.scalar.activation(nrm[0:rcw, :], ss[0:rcw, :], AF.Sqrt)
                nc.vector.tensor_scalar(
                    nrm[0:rcw, :], nrm[0:rcw, :], float(n_mean) * 1e-12, None, OP.max
                )
                rn = etmp.tile([128, 1], f32, name="rn", tag="rn")
                nc.vector.reciprocal(rn[0:rcw, :], nrm[0:rcw, :])
                ov = outp.tile([128, D], f32, name="ov", tag="ov")
                nc.vector.tensor_scalar(
                    ov[0:rcw, :], amv, rn[0:rcw, :], None, OP.mult
                )
                nc.scalar.dma_start(
                    out=out_dram[t][rc * 128 : rc * 128 + rcw, :], in_=ov[0:rcw, :]
                )

    _split_sync_waits(nc)
    return nc


def host_prep(inputs):
    """Shared (replicated) device inputs + per-core shards."""
    shared = {}
    for t in TYPES:
        X = inputs[FEAT_KEY[t]]
        S, SP = TYPE_N[t], TYPE_SPAD[t]
        xa = np.zeros((D + 1, SP), np.float32)
        xa[0:D, 0:S] = np.asarray(X, np.float32).T
        xa[D, 0:S] = 1.0
        shared[f"xaugt_{t}"] = xa
        blocks = []
        for r in SRC_PACKS[t]:
            wb = np.empty((D + 1, D), np.float32)
            wb[0:D] = np.asarray(inputs["W_" + r], np.float32).T
            wb[D] = np.asarray(inputs["b_" + r], np.float32)
            blocks.append(wb)
        shared[f"wpack_{t}"] = np.ascontiguousarray(np.concatenate(blocks, axis=1))

    in_maps = []
    for c in range(N_CORES):
        m = dict(shared)
        for r in MAIN_ORDER:
            A = inputs["A_" + r]
            S, SP = TYPE_N[REL_SRC[r]], TYPE_SPAD[REL_SRC[r]]
            R = TYPE_N[REL_DEST[r]] // N_CORES
            at = np.zeros((SP, R), np.float16)
            at[0:S] = A[c * R : (c + 1) * R, :].T
            # swizzle: [SP, R] -> [SP/512, 128, 4*R] so each SBUF
            # partition's dma run is 4 rows = 4KB contiguous
            m[f"at_{r}"] = np.ascontiguousarray(
                at.reshape(SP // 512, 4, 128, R).transpose(0, 2, 1, 3)
            ).reshape(SP // 512, 128, 4 * R)
        for t in TYPES:
            R = TYPE_N[t] // N_CORES
            F = inputs[FEAT_KEY[t]]
            m[f"featd_{t}"] = np.ascontiguousarray(
                np.asarray(F, np.float32)[c * R : (c + 1) * R, :]
            )
        in_maps.append(m)
    return in_maps


_NC_CACHE = None


def _get_nc():
    global _NC_CACHE
    if _NC_CACHE is None:
        _NC_CACHE = build_nc()
    return _NC_CACHE


def kernel(**inputs):
    from concourse.bass_utils import run_bass_kernel_spmd

    nc = _get_nc()
    in_maps = host_prep(inputs)

    kwargs = {}
    if os.environ.get("TRN_KERNEL_TRACE"):
        import tempfile

        kwargs["trace"] = True
        tmpdir = os.environ.get("TRN_KERNEL_TRACE_DIR")
        if tmpdir:
            os.makedirs(tmpdir, exist_ok=True)
            kwargs["tmpdir"] = tempfile.mkdtemp(prefix="run_", dir=tmpdir)
            print("trace dir:", kwargs["tmpdir"])

    res = run_bass_kernel_spmd(
        nc, in_maps, core_ids=list(range(N_CORES)), **kwargs
    )
    if os.environ.get("TRN_KERNEL_TRACE"):
        kernel.last_exec_time_ns = res.exec_time_ns

    outs = []
    for t in ["drug", "protein", "sideeffect", "disease"]:
        outs.append(
            np.concatenate([res.results[c][f"out_{t}"] for c in range(N_CORES)], axis=0)
        )
    return tuple(outs)
